# revision 1
# baseline (speedup 1.0000x reference)
"""Trainium2 kernel for ApplyStickerLayer: out = roll(subimg, (80,80), (2,3)) + base_image.

Input structure (guaranteed by the layer): subimg is zero outside the 50x50
sticker at the origin, and base_image is zero inside the 50x50 destination
window at (80,80).  The roll therefore just moves the sticker into the window:

    out[b] = base            everywhere except the window [80:130, 80:130]
    out[b][win] = subimg[b][0:50, 0:50] + base[win]

Pure data parallel across 8 NeuronCores (32 batches per core).  Per core the
NEFF writes each (b, c) channel image as four disjoint DRAM regions in the
flattened 50176-element channel space:

    top   [0, 18000)                 rows 0..79 + row 80 cols 0..79
    diag  49 x 174 strided chunks    rows 80..129 minus the window columns
    tail  [29026, 50176)             row 129 cols 130.. + rows 130..223
    win   50 x 50                    sticker + base-window

All regions are disjoint so there are no write-ordering hazards.

Implementation notes (hardware-measured):
  * SWDGE (nc.gpsimd) spreads descriptors across all 16 SDMA engines;
    HWDGE-dynamic (nc.sync/nc.scalar) concentrates on ~2 engines per ring at
    ~50ns/descriptor -> useless for bulk.  Everything goes through SWDGE.
  * SWDGE cannot encode stride-0 (broadcast) sources (runtime crash), so the
    base pieces are physically replicated REP times in SBUF (log-doubling
    SBUF->SBUF copies), and each store then covers REP batches x 3 channels
    with one regular-stride 3D access pattern: the batch*channel dimension
    merges because channel images are contiguous per batch in DRAM.

Traffic per core: ~19.3 MB written, ~1.6 MB read -> memory roofline ~55-60 us.
"""

import sys

import numpy as np

if "/opt/trn_rl_repo" not in sys.path:
    sys.path.insert(0, "/opt/trn_rl_repo")

import concourse.bacc as bacc
import concourse.bass as bass
import concourse.mybir as mybir
import concourse.tile as tile
from concourse.bass_utils import run_bass_kernel_spmd

N_CORES = 8
B, C, H, W = 256, 3, 224, 224
BS = B // N_CORES  # 32 batches per core
SH, SW = 80, 80  # roll shift == window origin
KH, KW = 50, 50  # sticker size

CHW = H * W  # 50176 elements per channel image
IMG = C * CHW  # 150528 elements per batch image

TOP_LEN = SH * W + SW  # 18000
DIAG_OFF = TOP_LEN + KW  # 18050
DIAG_ROWS = KH - 1  # 49
DIAG_LEN = W - KW  # 174
TAIL_OFF = DIAG_OFF + (DIAG_ROWS - 1) * W + DIAG_LEN + KW  # 29026
TAIL_LEN = CHW - TAIL_OFF  # 21150

_F32 = mybir.dt.float32

DEFAULT_CFG = {
    "top": (25, 720),  # p x f factorization of TOP_LEN
    "tail": (45, 470),  # p x f factorization (avoid p=47/94: SWDGE ucode bug)
    "diag": (49, 174),  # fixed by geometry
    "rep": 8,  # base pieces replicated this many times in SBUF
    "do_pieces": True,
    "swq": 1,  # num_swdge_queues
    "sc": None,  # batches per store DMA (default: rep)
    "kinds": ("top", "diag", "tail"),
    "do_win": True,
}


def _piece_src_ap(base, o, kind, p, f):
    """DRAM source AP for one piece of all 3 channels, walk order (p, c, f)."""
    if kind == "diag":
        return bass.AP(base, o, [[W, p], [CHW, C], [1, f]])
    return bass.AP(base, o, [[f, p], [CHW, C], [1, f]])


def build_nc(cfg=None):
    cfg = {**DEFAULT_CFG, **(cfg or {})}
    rep = cfg["rep"]
    assert BS % rep == 0
    pieces = [
        ("top", 0, *cfg["top"]),
        ("diag", DIAG_OFF, *cfg["diag"]),
        ("tail", TAIL_OFF, *cfg["tail"]),
    ]
    assert cfg["top"][0] * cfg["top"][1] == TOP_LEN
    assert cfg["tail"][0] * cfg["tail"][1] == TAIL_LEN
    assert tuple(cfg["diag"]) == (DIAG_ROWS, DIAG_LEN)

    nc = bacc.Bacc(
        "TRN2",
        target_bir_lowering=False,
        num_devices=N_CORES,
        num_swdge_queues=cfg["swq"],
    )
    sub = nc.declare_dram_parameter("subimg", [BS, C, H, W], _F32, isOutput=False)
    base = nc.declare_dram_parameter("base", [C, H, W], _F32, isOutput=False)
    out = nc.declare_dram_parameter("out", [BS, C, H, W], _F32, isOutput=True)

    with tile.TileContext(nc) as tc:
        with (
            tc.tile_pool(name="consts", bufs=1) as cpool,
            tc.tile_pool(name="work", bufs=1) as wpool,
        ):
            # ---- load base pieces (3 channels concatenated), replicate x rep ----
            for kind, o, p, f in [
                pc for pc in pieces if cfg["do_pieces"] and pc[0] in cfg["kinds"]
            ]:
                cf = C * f  # one replica section: (p, (c, f))
                t = cpool.tile([p, rep * cf], _F32, tag=kind)
                nc.gpsimd.dma_start(
                    out=t[:, 0:cf].rearrange("p (c f) -> p c f", c=C),
                    in_=_piece_src_ap(base, o, kind, p, f),
                )
                k = 1
                while k < rep:  # log-doubling replication in the free dim
                    k2 = min(2 * k, rep)
                    nc.gpsimd.dma_start(
                        out=t[:, k * cf : k2 * cf], in_=t[:, 0 : (k2 - k) * cf]
                    )
                    k = k2

                # ---- stores: each covers sc batches x 3 channels ----
                sc = cfg["sc"] or rep
                assert rep % sc == 0 or sc % rep == 0
                sc = min(sc, rep) if sc <= rep else sc
                step0 = [W, p] if kind == "diag" else [f, p]
                for b0 in range(0, BS, sc):
                    dst = bass.AP(
                        out, b0 * IMG + o, [step0, [CHW, sc * C], [1, f]]
                    )
                    nc.gpsimd.dma_start(
                        out=dst,
                        in_=t[:, 0 : sc * cf].rearrange("p (bc f) -> p bc f", f=f),
                    )

            if cfg["do_win"]:
                # ---- window path: win = sticker + base_window, all via DMA ----
                t_bwin = cpool.tile([KH, C * KW], _F32, tag="bwin")
                nc.gpsimd.dma_start(
                    out=t_bwin[:, :].rearrange("p (c w) -> p c w", c=C),
                    in_=base[:, SH : SH + KH, SW : SW + KW].rearrange("c h w -> h c w"),
                )
                t_win = wpool.tile([KH, BS * C * KW], _F32, tag="win")
                nc.gpsimd.dma_start(out=t_win[:, 0 : C * KW], in_=t_bwin[:, :])
                k = 1
                while k < BS:  # replicate base window across batches
                    k2 = min(2 * k, BS)
                    nc.gpsimd.dma_start(
                        out=t_win[:, k * C * KW : k2 * C * KW],
                        in_=t_win[:, 0 : (k2 - k) * C * KW],
                    )
                    k = k2
                # accumulate the sticker during its load (SWDGE accum)
                nc.gpsimd.dma_start(
                    out=t_win[:, :].rearrange("p (bc w) -> p bc w", w=KW),
                    in_=sub[:, :, 0:KH, 0:KW].rearrange("b c h w -> h b c w"),
                    accum_op=mybir.AluOpType.add,
                )
                nc.gpsimd.dma_start(
                    out=out[:, :, SH : SH + KH, SW : SW + KW].rearrange(
                        "b c h w -> h b c w"
                    ),
                    in_=t_win[:, :].rearrange("p (bc w) -> p bc w", w=KW),
                )
    nc.compile()
    return nc


def run(inputs, cfg=None, trace=False, **kw):
    sub = np.ascontiguousarray(inputs["subimg"], dtype=np.float32)
    basei = np.ascontiguousarray(inputs["base_image"], dtype=np.float32)
    assert sub.shape == (B, C, H, W) and basei.shape == (1, C, H, W)

    nc = build_nc(cfg)
    in_maps = [
        {"subimg": sub[i * BS : (i + 1) * BS], "base": basei[0]}
        for i in range(N_CORES)
    ]
    res = run_bass_kernel_spmd(nc, in_maps, list(range(N_CORES)), trace=trace, **kw)
    full = np.concatenate(
        [res.results[i]["out"] for i in range(N_CORES)], axis=0
    ).astype(np.float32, copy=False)
    return full, res


def kernel(**inputs) -> np.ndarray:
    out, _ = run(inputs)
    return out



# revision 4
# speedup vs baseline: 1.1961x; 1.1961x over previous
"""Trainium2 kernel for ApplyStickerLayer: out = roll(subimg, (80,80), (2,3)) + base_image.

Structure (guaranteed by the layer): subimg is zero outside the 50x50 sticker
at the origin, base_image is zero inside the destination window, and the roll
never wraps -- so per (b, c) channel image (flat, 50176 elems):

    out[b,c] = base[c] + shift_by_18000(sub[b,c])

and the image splits into three row bands:

    TOP  rows   0.. 79  flat [    0, 17920)  pure base
    MID  rows  80..129  flat [17920, 29120)  base + shifted sticker  (per batch)
    BOT  rows 130..223  flat [29120, 50176)  pure base

TOP/BOT are identical across batches: store them straight out of a shared SBUF
copy of base (replicated REP x in the free dim so one store covers REP batches
x 3 channels -- SWDGE can't do stride-0 sources).  Every store is a fat
contiguous run (>= 4.4 KB per descriptor), which matters because SWDGE
descriptors cost ~150 ns each regardless of size.

MID is composed on-chip with one TensorE pass per column chunk:

    psum[96, f] = W.T @ x      W [99, 96] = [channel selector ; identity]
                               x [99, f]  = [3 base mid rows ; 96 sub rows]

The sub rows are DMA'd into x with a +80 column offset, which lands the
sticker at columns 80..129 of each row (the spill-over is all zeros by the
sparsity guarantee).  PSUM chunks are copied to SBUF by DVE and stored as
96 x 22 KB descriptors.

Per core: ~19.3 MB written + ~5 MB read => HBM roofline ~68 us.
"""

import sys

import numpy as np

if "/opt/trn_rl_repo" not in sys.path:
    sys.path.insert(0, "/opt/trn_rl_repo")

import concourse.bacc as bacc
import concourse.bass as bass
import concourse.mybir as mybir
import concourse.tile as tile
from concourse.bass_utils import run_bass_kernel_spmd

N_CORES = 8
B, C, H, W = 256, 3, 224, 224
BS = B // N_CORES  # 32 batches per core
BC = BS * C  # 96 (batch, channel) images per core
SH, SW = 80, 80
KH, KW = 50, 50

CHW = H * W  # 50176
IMG = C * CHW  # 150528
SHIFT = SH * W + SW  # 18000: the roll as a flat shift

TOP_OFF, TOP_LEN = 0, SH * W  # [0, 17920)
MID_OFF, MID_LEN = SH * W, KH * W  # [17920, 29120)
BOT_OFF = MID_OFF + MID_LEN  # 29120
BOT_LEN = CHW - BOT_OFF  # 21056

_F32 = mybir.dt.float32

DEFAULT_CFG = {
    "p_top": 16,  # partition dim of the shared TOP tile (17920 % p == 0)
    "p_bot": 16,  # partition dim of the shared BOT tile (21056 % p == 0)
    "rep": 4,  # base TOP/BOT replicas in SBUF -> batches covered per store
    "n_pass": 2,  # MID column passes (x/S load granularity)
    "mm_f": 512,  # matmul free-dim chunk (<= 512, one PSUM bank)
    "bout_f": 2800,  # MID store chunk (multiple per pass; MID_LEN/n_pass % bout_f == 0)
    "bout_bufs": 2,
    "psum_bufs": 4,
    "swq": 1,  # num_swdge_queues
}


def build_nc(cfg=None):
    cfg = {**DEFAULT_CFG, **(cfg or {})}
    p_top, p_bot, rep = cfg["p_top"], cfg["p_bot"], cfg["rep"]
    f_top, f_bot = TOP_LEN // p_top, BOT_LEN // p_bot
    assert f_top * p_top == TOP_LEN and f_bot * p_bot == BOT_LEN
    assert BS % rep == 0
    n_pass = cfg["n_pass"]
    pass_len = MID_LEN // n_pass
    assert pass_len * n_pass == MID_LEN
    bout_f = cfg["bout_f"]
    assert pass_len % bout_f == 0
    mm_f = cfg["mm_f"]
    K = C + BC  # 99: matmul contraction (3 base rows + 96 sub rows)

    nc = bacc.Bacc(
        "TRN2",
        target_bir_lowering=False,
        num_devices=N_CORES,
        num_swdge_queues=cfg["swq"],
    )
    sub = nc.declare_dram_parameter("subimg", [BS, C, H, W], _F32, isOutput=False)
    base = nc.declare_dram_parameter("base", [C, H, W], _F32, isOutput=False)
    wsel = nc.declare_dram_parameter("wsel", [K, BC], _F32, isOutput=False)
    out = nc.declare_dram_parameter("out", [BS, C, H, W], _F32, isOutput=True)

    with tile.TileContext(nc) as tc:
        with (
            tc.tile_pool(name="consts", bufs=1) as cpool,
            tc.tile_pool(name="bout", bufs=cfg["bout_bufs"]) as bpool,
            tc.tile_pool(name="psum", bufs=cfg["psum_bufs"], space=bass.MemorySpace.PSUM) as ppool,
        ):
            # ---- shared TOP/BOT base tiles, replicated rep x in the free dim ----
            shared = []  # (tile, band_off, p, f)
            for tag, off, p, f in (
                ("top", TOP_OFF, p_top, f_top),
                ("bot", BOT_OFF, p_bot, f_bot),
            ):
                cf = C * f
                t = cpool.tile([p, rep * cf], _F32, tag=tag)
                nc.gpsimd.dma_start(
                    out=t[:, 0:cf].rearrange("p (c f) -> p c f", c=C),
                    in_=bass.AP(base, off, [[f, p], [CHW, C], [1, f]]),
                )
                k = 1
                while k < rep:
                    k2 = min(2 * k, rep)
                    nc.gpsimd.dma_start(
                        out=t[:, k * cf : k2 * cf], in_=t[:, 0 : (k2 - k) * cf]
                    )
                    k = k2
                shared.append((t, off, p, f))

            # ---- matmul weights: [channel selector ; identity] ----
            t_w = cpool.tile([K, BC], _F32, tag="w")
            nc.gpsimd.dma_start(out=t_w[:, :], in_=wsel[:, :])

            # ---- TOP/BOT stores: one per rep-batch group per band ----
            for t, off, p, f in shared:
                for b0 in range(0, BS, rep):
                    dst = bass.AP(
                        out, b0 * IMG + off, [[f, p], [CHW, rep * C], [1, f]]
                    )
                    nc.gpsimd.dma_start(
                        out=dst,
                        in_=t[:, :].rearrange("p (rc f) -> p rc f", f=f),
                    )

            # ---- MID: per pass, load base+sub into x, matmul, evac, store ----
            t_x = cpool.tile([K, SW + pass_len], _F32, tag="x")
            for ps in range(n_pass):
                c0 = ps * pass_len  # column offset inside the MID band
                # base mid rows -> x[96:99, 0:pass_len]
                nc.gpsimd.dma_start(
                    out=t_x[BC:K, 0:pass_len],
                    in_=bass.AP(base, MID_OFF + c0, [[CHW, C], [1, pass_len]]),
                )
                # sub rows (partitions 0..95), shifted +80 columns
                if ps == 0:
                    # x[bc, j] = sub[bc, j - 80]; j < 80 has no source -> zero
                    nc.vector.memset(t_x[0:BC, 0:SW], 0.0)
                    nc.gpsimd.dma_start(
                        out=t_x[0:BC, SW : SW + pass_len],
                        in_=bass.AP(sub, 0, [[CHW, BC], [1, pass_len]]),
                    )
                else:
                    nc.gpsimd.dma_start(
                        out=t_x[0:BC, 0:pass_len],
                        in_=bass.AP(sub, c0 - SW, [[CHW, BC], [1, pass_len]]),
                    )
                for f0 in range(0, pass_len, bout_f):
                    t_b = bpool.tile([BC, bout_f], _F32, tag="bout")
                    for m0 in range(0, bout_f, mm_f):
                        mf = min(mm_f, bout_f - m0)
                        t_p = ppool.tile([BC, mm_f], _F32, tag="psum")
                        nc.tensor.matmul(
                            t_p[:, 0:mf],
                            t_w[:, :],
                            t_x[:, f0 + m0 : f0 + m0 + mf],
                        )
                        nc.vector.tensor_copy(t_b[:, m0 : m0 + mf], t_p[:, 0:mf])
                    nc.gpsimd.dma_start(
                        out=bass.AP(
                            out, MID_OFF + c0 + f0, [[CHW, BC], [1, bout_f]]
                        ),
                        in_=t_b[:, :],
                    )
    nc.compile()
    return nc


def _make_wsel():
    K = C + BC
    w = np.zeros((K, BC), dtype=np.float32)
    for bc in range(BC):
        w[bc, bc] = 1.0  # identity for the shifted sub rows (partitions 0..95)
        w[BC + bc % C, bc] = 1.0  # base channel selector (partitions 96..98)
    return w


def run(inputs, cfg=None, trace=False, **kw):
    sub = np.ascontiguousarray(inputs["subimg"], dtype=np.float32)
    basei = np.ascontiguousarray(inputs["base_image"], dtype=np.float32)
    assert sub.shape == (B, C, H, W) and basei.shape == (1, C, H, W)

    nc = build_nc(cfg)
    w = _make_wsel()
    in_maps = [
        {"subimg": sub[i * BS : (i + 1) * BS], "base": basei[0], "wsel": w}
        for i in range(N_CORES)
    ]
    res = run_bass_kernel_spmd(nc, in_maps, list(range(N_CORES)), trace=trace, **kw)
    full = np.concatenate(
        [res.results[i]["out"] for i in range(N_CORES)], axis=0
    ).astype(np.float32, copy=False)
    return full, res


def kernel(**inputs) -> np.ndarray:
    out, _ = run(inputs)
    return out
